# revision 1
# baseline (speedup 1.0000x reference)
"""BitLinear 1.58 (nn_BitLinear158) Trainium2 Bass kernel.

Problem: x:[4,2048,4096] f32, weight:[4096,4096] f32 ->
         absmax-group-quantized x (8-bit fake quant, groups of 64) @
         ternary-quantized weight.T (per-row absmean scale) -> [4,2048,4096].

Sharding: data-parallel over tokens. Each of the 8 cores takes 1024 tokens
and the full weight; outputs concatenate along tokens. This replicates the
(cheap) weight pipeline but minimizes DMA+vector work versus sharding
out_features: x-quant is 8x smaller per core and HBM traffic per core is
96MiB vs 152MiB.

Per-core kernel:
  - activation quant: absmax per (token, 64-group), scale=127*recip(absmax),
    q=rint(x*scale) via the +/-1.5*2^23 trick (round-half-even, matching
    jnp.round), x_q=q*(absmax/127) cast to fp16, transposed on the fly
    (xbar dma transpose) into the matmul-stationary layout [128,K/128,M].
  - weight ternarize: s=max(mean|row|,eps) with a two-stage compensated
    reduction (group sums, then an exact 2^-12-grid split so the final
    accumulation is error-free: my s is closer to the true mean than any
    f32 single-pass sum, minimizing disagreement with the f32 reference
    at the discontinuous round(w/s) boundaries); ternary values computed
    as exact comparisons t = (w > 0.5s) - (w < -0.5s), which equals
    clip(round-half-even(w/s),-1,1) for all non-boundary w and avoids any
    divide/round rounding concerns. t is exact in fp16.
  - matmul: psum[m,o] += xq_t[:,ks,m-block].T @ t_t[:,ks,o-tile] over the
    32 contraction chunks, fp16 operands, fp32 psum.
  - eviction: out = psum * s_row (per-column broadcast of s).
"""
import sys

sys.path.insert(0, "/opt/trn_rl_repo")

import numpy as np

B, S, D_IN, D_OUT = 4, 2048, 4096, 4096
N_CORES = 8
M_TOT = B * S
M_C = M_TOT // N_CORES

P = 128
G = 64
OT = 256                        # columns per o-tile (psum free dim)
MAGIC = float(1.5 * 2.0 ** 23)  # fp32 round-to-nearest-even trick
MAGIC2 = float(1.5 * 2.0 ** 11)  # quantize-to-2^-12-grid trick
EPS = 1e-5
QMAX = 127.0
INV_QMAX = float(np.float32(1.0 / 127.0))

_cache = {}


def _build(M, K, O):
    import concourse.bass as bass
    import concourse.tile as tile
    from concourse import bacc, mybir

    f32 = mybir.dt.float32
    f16 = mybir.dt.float16
    bf16 = mybir.dt.bfloat16
    Alu = mybir.AluOpType
    Act = mybir.ActivationFunctionType
    Ax = mybir.AxisListType

    K2 = K // 2
    KSUB = K // P
    MB = M // P
    NOT = O // OT
    OSUB = OT // P
    NG = K // G
    NG2 = NG // 2
    FIXC = float(1.5 * 2.0 ** 24)  # f32 grid-of-2 round (ties-to-even)

    nc = bacc.Bacc("TRN2", target_bir_lowering=False, num_devices=1)
    x = nc.dram_tensor("x", [M, K], f32, kind="ExternalInput")
    w = nc.dram_tensor("w", [O, K], f32, kind="ExternalInput")
    out = nc.dram_tensor("out", [M, O], f32, kind="ExternalOutput")
    s_scr = nc.dram_tensor("s_scr", [O, 1], f32, kind="Internal")

    xap, wap, oap = x.ap(), w.ap(), out.ap()

    with tile.TileContext(nc) as tc:
        with (
            tc.tile_pool(name="xq", bufs=1) as xq_pool,
            tc.tile_pool(name="stage", bufs=8) as stage,
            tc.tile_pool(name="sg", bufs=2) as sg_pool,
            tc.tile_pool(name="tt", bufs=2) as tt_pool,
            tc.tile_pool(name="small", bufs=3) as small,
            tc.tile_pool(name="ev", bufs=4) as ev_pool,
            tc.tile_pool(name="sb", bufs=2) as sb_pool,
            tc.tile_pool(name="ps", bufs=8, space="PSUM") as ps_pool,
        ):
            tt_tiles = {}
            w_stage = {}

            def wload(ot):
                """Issue the (wait-free) weight loads for o-tile ot."""
                tiles = []
                for osub in range(OSUB):
                    o0 = ot * OT + osub * P
                    for h in range(2):
                        wh = stage.tile([P, K2], f32, tag="stage",
                                        name=f"wh{ot}_{osub}_{h}")
                        nc.sync.dma_start(
                            wh[:], wap[o0:o0 + P, h * K2:(h + 1) * K2])
                        tiles.append(wh)
                w_stage[ot] = tiles

            def wcompute(ot):
                """Ternarize OT weight rows into tt_tiles[ot] (bf16 2t)."""
                ttl = tt_pool.tile([P, KSUB, OT], bf16, tag="tt",
                                   name=f"tt{ot}")
                tt_tiles[ot] = ttl
                staged = w_stage.pop(ot)
                for osub in range(OSUB):
                    o0 = ot * OT + osub * P
                    gs = small.tile([P, NG], f32, tag="gs")
                    whs = staged[osub * 2:osub * 2 + 2]
                    for h in range(2):
                        nc.vector.tensor_reduce(
                            gs[:, h * NG2:(h + 1) * NG2],
                            whs[h].rearrange("p (g e) -> p g e", e=G),
                            Ax.X, Alu.add, apply_absolute_value=True)
                    # s = max(mean|row|, eps), two-stage compensated sum
                    hq = small.tile([P, NG], f32, tag="hq")
                    nc.vector.tensor_scalar(hq[:], gs[:], MAGIC2, MAGIC2,
                                            Alu.add, Alu.subtract)
                    lq = small.tile([P, NG], f32, tag="lq")
                    nc.vector.tensor_tensor(lq[:], gs[:], hq[:], Alu.subtract)
                    sh = small.tile([P, 1], f32, tag="sh")
                    nc.vector.tensor_reduce(sh[:], hq[:], Ax.X, Alu.add)
                    sl = small.tile([P, 1], f32, tag="sl")
                    nc.vector.tensor_reduce(sl[:], lq[:], Ax.X, Alu.add)
                    ssum = small.tile([P, 1], f32, tag="ssum")
                    nc.vector.tensor_tensor(ssum[:], sh[:], sl[:], Alu.add)
                    sv = small.tile([P, 1], f32, tag="sv")
                    nc.vector.tensor_scalar(sv[:], ssum[:],
                                            float(np.float32(1.0 / K)),
                                            EPS, Alu.mult, Alu.max)
                    bp = small.tile([P, 1], f32, tag="bp")
                    nc.vector.tensor_scalar(bp[:], sv[:], 0.5, None, Alu.mult)
                    bn = small.tile([P, 1], f32, tag="bn")
                    nc.vector.tensor_scalar(bn[:], sv[:], -0.5, None,
                                            Alu.mult)
                    # eviction scale is 0.5*s (the sign-sum below is 2t)
                    nc.sync.dma_start(s_scr.ap()[o0:o0 + P, :], bp[:])
                    # 2t = sign(w-0.5s) + sign(w+0.5s); exact comparisons.
                    # Boundary |w|==0.5s gives +-1; the f32 grid-of-2 magic
                    # round maps it to 0 (= round-half-even of w/s).
                    for h in range(2):
                        sga = sg_pool.tile([P, K2], bf16, tag="sga",
                                           name=f"sga{ot}_{osub}_{h}")
                        nc.scalar.activation(out=sga[:], in_=whs[h][:],
                                             func=Act.Sign, bias=bn[:],
                                             scale=1.0)
                        sgb = sg_pool.tile([P, K2], bf16, tag="sgb",
                                           name=f"sgb{ot}_{osub}_{h}")
                        nc.scalar.activation(out=sgb[:], in_=whs[h][:],
                                             func=Act.Sign, bias=bp[:],
                                             scale=1.0)
                        nc.vector.tensor_tensor(sga[:], sga[:], sgb[:],
                                                Alu.add)
                        nc.vector.tensor_scalar(sga[:], sga[:], FIXC, FIXC,
                                                Alu.add, Alu.subtract)
                        nc.sync.dma_start_transpose(
                            ttl[:, h * (KSUB // 2):(h + 1) * (KSUB // 2),
                                osub * P:(osub + 1) * P], sga[:])

            # prefetch first two o-tiles of weights; ternarize the first
            wload(0)
            wload(1)
            wcompute(0)

            # -------- activation quantization + transpose --------
            xq_tiles = [xq_pool.tile([P, KSUB, P], f16, tag=f"xq{mb}",
                                     name=f"xq{mb}") for mb in range(MB)]
            for mb in range(MB):
                for h in range(2):
                    xt = stage.tile([P, K2], f32, tag="stage",
                                    name=f"xt{mb}_{h}")
                    nc.sync.dma_start(
                        xt[:], xap[mb * P:(mb + 1) * P, h * K2:(h + 1) * K2])
                    xg = xt.rearrange("p (g e) -> p g e", e=G)
                    am = small.tile([P, NG2], f32, tag="am")
                    nc.vector.tensor_reduce(am[:], xg, Ax.X, Alu.max,
                                            apply_absolute_value=True)
                    am2 = small.tile([P, NG2], f32, tag="am2")
                    nc.vector.tensor_scalar(am2[:], am[:], EPS, None, Alu.max)
                    rc = small.tile([P, NG2], f32, tag="rc")
                    nc.vector.reciprocal(rc[:], am2[:])
                    scale = small.tile([P, NG2], f32, tag="scale")
                    nc.vector.tensor_scalar(scale[:], rc[:], QMAX, None,
                                            Alu.mult)
                    inv = small.tile([P, NG2], f32, tag="inv")
                    nc.vector.tensor_scalar(inv[:], am2[:], INV_QMAX, None,
                                            Alu.mult)
                    nc.vector.tensor_tensor(
                        xg, xg, scale[:, :, None].to_broadcast((P, NG2, G)),
                        Alu.mult)
                    nc.vector.tensor_scalar(xt[:], xt[:], MAGIC, MAGIC,
                                            Alu.add, Alu.subtract)
                    xq16 = sg_pool.tile([P, K2], f16, tag="sga",
                                        name=f"xq16_{mb}_{h}")
                    nc.vector.tensor_tensor(
                        xq16.rearrange("p (g e) -> p g e", e=G), xg,
                        inv[:, :, None].to_broadcast((P, NG2, G)), Alu.mult)
                    nc.scalar.dma_start_transpose(
                        xq_tiles[mb][:, h * (KSUB // 2):(h + 1) * (KSUB // 2),
                                     :], xq16[:])

            # -------- matmul + eviction, prefetching W two o-tiles ahead ----
            for ot in range(NOT):
                if ot + 2 < NOT:
                    wload(ot + 2)
                if ot + 1 < NOT:
                    wcompute(ot + 1)
                ttl = tt_tiles.pop(ot)
                s_base = s_scr.ap()[ot * OT:(ot + 1) * OT, 0]
                s_bc_ap = bass.AP(tensor=s_base.tensor, offset=s_base.offset,
                                  ap=[[0, P], *s_base.ap])
                sbc = sb_pool.tile([P, OT], f32)
                nc.gpsimd.dma_start(sbc[:], s_bc_ap)

                for mb in range(MB):
                    ps = ps_pool.tile([P, OT], f32)
                    for ks in range(KSUB):
                        nc.tensor.matmul(
                            ps[:], xq_tiles[mb][:, ks, :], ttl[:, ks, :],
                            start=(ks == 0), stop=(ks == KSUB - 1))
                    ev = ev_pool.tile([P, OT], f32)
                    nc.vector.tensor_tensor(ev[:], ps[:], sbc[:], Alu.mult)
                    nc.gpsimd.dma_start(
                        oap[mb * P:(mb + 1) * P, ot * OT:(ot + 1) * OT],
                        ev[:])

    nc.compile()
    return nc


def _get_nc():
    if "nc" not in _cache:
        _cache["nc"] = _build(M_C, D_IN, D_OUT)
    return _cache["nc"]


def run(x, weight, trace=False):
    """Run on 8 NeuronCores; returns (full output [B,S,D_OUT], results obj)."""
    from concourse.bass_utils import run_bass_kernel_spmd

    x = np.ascontiguousarray(np.asarray(x, dtype=np.float32))
    w = np.ascontiguousarray(np.asarray(weight, dtype=np.float32))
    assert x.shape == (B, S, D_IN) and w.shape == (D_OUT, D_IN)
    xf = x.reshape(M_TOT, D_IN)
    nc = _get_nc()
    in_maps = [
        {"x": np.ascontiguousarray(xf[c * M_C:(c + 1) * M_C]), "w": w}
        for c in range(N_CORES)
    ]
    res = run_bass_kernel_spmd(nc, in_maps, core_ids=list(range(N_CORES)),
                               trace=trace)
    outf = np.concatenate([res.results[c]["out"] for c in range(N_CORES)],
                          axis=0)
    return outf.reshape(B, S, D_OUT), res


def kernel(x, weight):
    out, _ = run(x, weight)
    return out



# revision 3
# speedup vs baseline: 1.2873x; 1.2873x over previous
"""BitLinear 1.58 (nn_BitLinear158) Trainium2 Bass kernel.

Problem: x:[4,2048,4096] f32, weight:[4096,4096] f32 ->
         absmax-group-quantized x (8-bit fake quant, groups of 64) @
         ternary-quantized weight.T (per-row absmean scale) -> [4,2048,4096].

Sharding: data-parallel over tokens. Each of the 8 cores takes 1024 tokens
and the full weight; outputs concatenate along tokens. This minimizes
replicated elementwise work (x-quant is 8x smaller per core than a
column-parallel split would need).

Per-core kernel (M=1024 tokens, K=4096, O=4096):
  - activation quant: absmax per (token, 64-group), scale=127*recip(absmax),
    q=rint(x*scale) via the +/-1.5*2^23 magic trick (round-half-even,
    matching jnp.round), x_q=q*(absmax/127) cast to fp16, dma-transposed
    into the matmul moving layout [128k, K/128, M].
  - weight ternarize: s=max(mean|row|,eps) with a two-stage compensated
    reduction (group sums, then an exact 2^-12-grid split so the final
    accumulation is error-free); ternary values via exact comparisons
    t = (w > 0.5s) - (w < -0.5s) = clip(round-half-even(w/s),-1,1) for all
    non-boundary w, fused into 2 DVE passes (tensor_scalar is_lt +
    scalar_tensor_tensor is_gt/subtract), fp16 out, dma-transposed into the
    stationary layout [128k, K/128, 128o].
  - matmul: stationary = ternary weights [k, o-block], moving = x_q [k, m]
    -> psum[o, m]. 32 contraction chunks accumulate per psum bank; psum
    free dim 512 (full f32 bank). The [o, m] output orientation makes the
    per-row weight scale a per-partition scalar.
  - eviction: Activation engine copy with scale=s (per-partition AP), so
    the vector engine does no psum work at all; output stored transposed
    [O, M] per core and untransposed on the host at gather time.
Engine budget: PE ~437us (roofline), DVE ~250us, Act ~100us, all DMA
queues < 300us -> PE-bound with a ~60us activation-quant head.
"""
import sys

sys.path.insert(0, "/opt/trn_rl_repo")

import numpy as np

B, S, D_IN, D_OUT = 4, 2048, 4096, 4096
N_CORES = 8
M_TOT = B * S
M_C = M_TOT // N_CORES

P = 128
G = 64
MAGIC = float(1.5 * 2.0 ** 23)   # fp32 round-to-nearest-even trick
MAGIC2 = float(1.5 * 2.0 ** 11)  # quantize-to-2^-12-grid trick
EPS = 1e-5
QMAX = 127.0
INV_QMAX = float(np.float32(1.0 / 127.0))

_cache = {}


def _build(M, K, O):
    import concourse.bass as bass
    import concourse.tile as tile
    from concourse import bacc, mybir

    f32 = mybir.dt.float32
    f16 = mybir.dt.float16
    Alu = mybir.AluOpType
    Act = mybir.ActivationFunctionType
    Ax = mybir.AxisListType

    K2 = K // 2          # 2048, half-row staged per DMA
    KSUB = K // P        # 32 contraction chunks
    KS2 = KSUB // 2      # 16 chunks per half
    MB = M // P          # 8 token blocks
    OC = O // P          # 32 out-feature blocks
    MH = M // 512        # 2 psum column halves
    NGX = K2 // G        # 32 quant groups per x half-tile
    NGW = K // G         # 64 abs-mean groups per w row

    nc = bacc.Bacc("TRN2", target_bir_lowering=False, num_devices=1)
    x = nc.dram_tensor("x", [M, K], f32, kind="ExternalInput")
    w = nc.dram_tensor("w", [O, K], f32, kind="ExternalInput")
    # transposed output [O, M]; host untransposes at gather time
    out = nc.dram_tensor("out", [O, M], f32, kind="ExternalOutput")

    xap, wap, oap = x.ap(), w.ap(), out.ap()

    with tile.TileContext(nc) as tc:
        with (
            tc.tile_pool(name="xq", bufs=1) as xq_pool,
            tc.tile_pool(name="xst", bufs=2) as xst,
            tc.tile_pool(name="xq16", bufs=2) as xq16_pool,
            tc.tile_pool(name="wst", bufs=6) as wst,
            tc.tile_pool(name="wneg", bufs=2) as wneg,
            tc.tile_pool(name="wt2", bufs=3) as wt2,
            tc.tile_pool(name="wt", bufs=3) as wt_pool,
            tc.tile_pool(name="small", bufs=3) as small,
            tc.tile_pool(name="sv", bufs=8) as sv_pool,
            tc.tile_pool(name="ev", bufs=4) as ev_pool,
            tc.tile_pool(name="ps", bufs=6, space="PSUM") as ps_pool,
        ):
            # -------- activation loads (SP queue, issued first) --------
            x_stage = {}
            for mb in range(MB):
                for h in range(2):
                    xt = xst.tile([P, K2], f32, tag="xst",
                                  name=f"xt{mb}_{h}")
                    nc.sync.dma_start(
                        xt[:], xap[mb * P:(mb + 1) * P, h * K2:(h + 1) * K2])
                    x_stage[(mb, h)] = xt

            # -------- weight loads (SP queue, 3-oc lookahead) --------
            w_stage = {}

            def wload(oc):
                tiles = []
                for h in range(2):
                    wh = wst.tile([P, K2], f32, tag="wst",
                                  name=f"wh{oc}_{h}")
                    nc.sync.dma_start(
                        wh[:], wap[oc * P:(oc + 1) * P, h * K2:(h + 1) * K2])
                    tiles.append(wh)
                w_stage[oc] = tiles

            for _oc in range(min(3, OC)):
                wload(_oc)

            # -------- activation quantization + transpose --------
            xq_t = xq_pool.tile([P, KSUB, M], f16, name="xq_t")
            for mb in range(MB):
                for h in range(2):
                    xt = x_stage.pop((mb, h))
                    xg = xt.rearrange("p (g e) -> p g e", e=G)
                    am = small.tile([P, NGX], f32, tag="am")
                    nc.vector.tensor_reduce(am[:], xg, Ax.X, Alu.max,
                                            apply_absolute_value=True)
                    am2 = small.tile([P, NGX], f32, tag="am2")
                    nc.vector.tensor_scalar(am2[:], am[:], EPS, None, Alu.max)
                    rc = small.tile([P, NGX], f32, tag="rc")
                    nc.vector.reciprocal(rc[:], am2[:])
                    scale = small.tile([P, NGX], f32, tag="scale")
                    nc.vector.tensor_scalar(scale[:], rc[:], QMAX, None,
                                            Alu.mult)
                    inv = small.tile([P, NGX], f32, tag="inv")
                    nc.vector.tensor_scalar(inv[:], am2[:], INV_QMAX, None,
                                            Alu.mult)
                    # xs = x * scale (group-broadcast), fused TSP form
                    nc.vector.scalar_tensor_tensor(
                        xg, xg, 0.0,
                        scale[:, :, None].to_broadcast((P, NGX, G)),
                        Alu.bypass, Alu.mult)
                    # q = rint(xs) via magic add/sub
                    nc.vector.tensor_scalar(xt[:], xt[:], MAGIC, MAGIC,
                                            Alu.add, Alu.subtract)
                    # x_q = q * (absmax/127) -> fp16
                    xq16 = xq16_pool.tile([P, K2], f16, tag="xq16",
                                          name=f"xq16_{mb}_{h}")
                    nc.vector.scalar_tensor_tensor(
                        xq16.rearrange("p (g e) -> p g e", e=G), xg, 0.0,
                        inv[:, :, None].to_broadcast((P, NGX, G)),
                        Alu.bypass, Alu.mult)
                    nc.scalar.dma_start_transpose(
                        xq_t[:, h * KS2:(h + 1) * KS2,
                             mb * P:(mb + 1) * P], xq16[:])

            # -------- weight ternarize --------
            wt_tiles = {}
            sv_tiles = {}

            def wternarize(oc):
                whs = w_stage.pop(oc)
                gs = small.tile([P, NGW], f32, tag="gs")
                for h in range(2):
                    nc.vector.tensor_reduce(
                        gs[:, h * NGX:(h + 1) * NGX],
                        whs[h].rearrange("p (g e) -> p g e", e=G),
                        Ax.X, Alu.add, apply_absolute_value=True)
                # s = max(mean|row|, eps), two-stage compensated sum
                hq = small.tile([P, NGW], f32, tag="hq")
                nc.vector.tensor_scalar(hq[:], gs[:], MAGIC2, MAGIC2,
                                        Alu.add, Alu.subtract)
                lq = small.tile([P, NGW], f32, tag="lq")
                nc.vector.tensor_tensor(lq[:], gs[:], hq[:], Alu.subtract)
                sh = small.tile([P, 1], f32, tag="sh")
                nc.vector.tensor_reduce(sh[:], hq[:], Ax.X, Alu.add)
                sl = small.tile([P, 1], f32, tag="sl")
                nc.vector.tensor_reduce(sl[:], lq[:], Ax.X, Alu.add)
                ssum = small.tile([P, 1], f32, tag="ssum")
                nc.vector.tensor_tensor(ssum[:], sh[:], sl[:], Alu.add)
                sv = sv_pool.tile([P, 1], f32, tag="sv", name=f"sv{oc}")
                nc.vector.tensor_scalar(sv[:], ssum[:],
                                        float(np.float32(1.0 / K)),
                                        EPS, Alu.mult, Alu.max)
                sv_tiles[oc] = sv
                bp = small.tile([P, 1], f32, tag="bp")
                nc.vector.tensor_scalar(bp[:], sv[:], 0.5, None, Alu.mult)
                bn = small.tile([P, 1], f32, tag="bn")
                nc.vector.tensor_scalar(bn[:], sv[:], -0.5, None, Alu.mult)
                # t = (w > 0.5s) - (w < -0.5s); exact comparisons, so the
                # boundary |w|==0.5s gives 0 (= round-half-even of +-0.5).
                wt = wt_pool.tile([P, KSUB, P], f16, tag="wt",
                                  name=f"wt{oc}")
                wt_tiles[oc] = wt
                for h in range(2):
                    neg = wneg.tile([P, K2], f16, tag="neg",
                                    name=f"neg{oc}_{h}")
                    nc.vector.tensor_scalar(neg[:], whs[h][:], bn[:], None,
                                            Alu.is_lt)
                    t2 = wt2.tile([P, K2], f16, tag="t2",
                                  name=f"t2_{oc}_{h}")
                    nc.vector.scalar_tensor_tensor(
                        t2[:], whs[h][:], bp[:], neg[:],
                        Alu.is_gt, Alu.subtract)
                    nc.scalar.dma_start_transpose(
                        wt[:, h * KS2:(h + 1) * KS2, :], t2[:])

            # -------- matmul + eviction --------
            for oc in range(OC):
                if oc + 3 < OC:
                    wload(oc + 3)
                wternarize(oc)
                wt = wt_tiles.pop(oc)
                sv = sv_tiles.pop(oc)
                for mh in range(MH):
                    ps = ps_pool.tile([P, 512], f32)
                    for ks in range(KSUB):
                        nc.tensor.matmul(
                            ps[:], wt[:, ks, :],
                            xq_t[:, ks, mh * 512:(mh + 1) * 512],
                            start=(ks == 0), stop=(ks == KSUB - 1))
                    ev = ev_pool.tile([P, 512], f32)
                    nc.scalar.activation(out=ev[:], in_=ps[:],
                                         func=Act.Copy, scale=sv[:])
                    nc.gpsimd.dma_start(
                        oap[oc * P:(oc + 1) * P, mh * 512:(mh + 1) * 512],
                        ev[:])

    nc.compile()
    return nc


def _get_nc():
    if "nc" not in _cache:
        _cache["nc"] = _build(M_C, D_IN, D_OUT)
    return _cache["nc"]


def run(x, weight, trace=False):
    """Run on 8 NeuronCores; returns (full output [B,S,D_OUT], results obj)."""
    from concourse.bass_utils import run_bass_kernel_spmd

    x = np.ascontiguousarray(np.asarray(x, dtype=np.float32))
    w = np.ascontiguousarray(np.asarray(weight, dtype=np.float32))
    assert x.shape == (B, S, D_IN) and w.shape == (D_OUT, D_IN)
    xf = x.reshape(M_TOT, D_IN)
    nc = _get_nc()
    in_maps = [
        {"x": np.ascontiguousarray(xf[c * M_C:(c + 1) * M_C]), "w": w}
        for c in range(N_CORES)
    ]
    res = run_bass_kernel_spmd(nc, in_maps, core_ids=list(range(N_CORES)),
                               trace=trace)
    outf = np.concatenate(
        [res.results[c]["out"].T for c in range(N_CORES)], axis=0)
    return np.ascontiguousarray(outf).reshape(B, S, D_OUT), res


def kernel(x, weight):
    out, _ = run(x, weight)
    return out


# revision 13
# speedup vs baseline: 1.3586x; 1.0553x over previous
"""BitLinear 1.58 (nn_BitLinear158) Trainium2 Bass kernel.

Problem: x:[4,2048,4096] f32, weight:[4096,4096] f32 ->
         absmax-group-quantized x (8-bit fake quant, groups of 64) @
         ternary-quantized weight.T (per-row absmean scale) -> [4,2048,4096].

Sharding: data-parallel over tokens (1024 tokens/core, full weight
replicated) — minimizes replicated elementwise work.

Per-core kernel (M=1024, K=4096, O=4096), engine-balanced so the tensor
engine (437us roofline at 2.4GHz) paces the pipeline:
  - PE: stationary = ternary weights [k,128o], moving = x_q [k,512m],
    psum [o,m]; 2048 matmuls stream at ~216ns each (full clock,
    ldweights pipelined).
  - DVE: the group reduces (x absmax, w abs-sum; reduces are DVE-only),
    the small scale chain, and the x round. ~300us.
  - Act (scalar): w loads (HWDGE), the two Sign passes per w half-tile
    (ternarize via sign(w-s/2)+sign(w+s/2), boundary fixed by a
    grid-of-2 magic round), psum eviction via Copy activation with
    scale=0.5*s as a per-partition AP (output orientation [o,m] makes
    the row scale per-partition). ~230us.
  - Pool (gpsimd): x scale-mult + dequant passes, w sign-sum combine +
    magic fix (USE_POOL), plus the SWDGE output stores. ~270us.
  - SP (sync): x loads + all xbar transposes (keeping transpose issue
    off the Act queue, which would head-of-line block behind evictions).
  - s computed with a two-stage compensated reduction (exact 2^-12-grid
    split) to track the f32 reference mean closely; ternary decisions
    are exact comparisons against +-0.5*s so there is no divide rounding.
"""
import sys

sys.path.insert(0, "/opt/trn_rl_repo")

import numpy as np

B, S, D_IN, D_OUT = 4, 2048, 4096, 4096
N_CORES = 8
M_TOT = B * S
M_C = M_TOT // N_CORES

P = 128
G = 64
MAGIC = float(1.5 * 2.0 ** 23)   # fp32 round-to-nearest-even trick
MAGIC2 = float(1.5 * 2.0 ** 11)  # quantize-to-2^-12-grid trick
FIXC = float(1.5 * 2.0 ** 24)    # f32 grid-of-2 round (ties-to-even)
EPS = 1e-5
QMAX = 127.0
INV_QMAX = float(np.float32(1.0 / 127.0))

# Note: gpsimd/Pool compute was measured and rejected — Pool TENSOR_SCALAR
# runs ~29us/[128,2048] (software DSP path) and even Pool TENSOR_TENSOR
# (~3.6us) degrades concurrent DVE throughput ~2x via SBUF port contention.

_cache = {}


def _build(M, K, O):
    import concourse.bass as bass
    import concourse.tile as tile
    from concourse import bacc, mybir

    f32 = mybir.dt.float32
    f16 = mybir.dt.float16
    Alu = mybir.AluOpType
    Act = mybir.ActivationFunctionType
    Ax = mybir.AxisListType

    K2 = K // 2          # 2048, half-row staged per DMA
    KSUB = K // P        # 32 contraction chunks
    KS2 = KSUB // 2      # 16 chunks per half
    MB = M // P          # token blocks
    OC = O // P          # out-feature blocks
    MH = M // 512        # psum column halves
    NGX = K2 // G        # 32 quant groups per x half-tile
    NGW = K // G         # 64 abs-mean groups per w row

    nc = bacc.Bacc("TRN2", target_bir_lowering=False, num_devices=1)
    x = nc.dram_tensor("x", [M, K], f32, kind="ExternalInput")
    w = nc.dram_tensor("w", [O, K], f32, kind="ExternalInput")
    # transposed output [O, M]; host untransposes at gather time
    out = nc.dram_tensor("out", [O, M], f32, kind="ExternalOutput")

    xap, wap, oap = x.ap(), w.ap(), out.ap()

    with tile.TileContext(nc) as tc:
        with (
            tc.tile_pool(name="xq", bufs=1) as xq_pool,
            tc.tile_pool(name="xst", bufs=2) as xst,
            tc.tile_pool(name="xq16", bufs=2) as xq16_pool,
            tc.tile_pool(name="wst", bufs=4) as wst,
            tc.tile_pool(name="sg", bufs=6) as sg_pool,
            tc.tile_pool(name="wt", bufs=3) as wt_pool,
            tc.tile_pool(name="small", bufs=3) as small,
            tc.tile_pool(name="sv", bufs=8) as sv_pool,
            tc.tile_pool(name="ev", bufs=4) as ev_pool,
            tc.tile_pool(name="ps", bufs=6, space="PSUM") as ps_pool,
        ):
            # -------- activation loads (SP queue, issued first) --------
            x_stage = {}
            for mb in range(MB):
                for h in range(2):
                    xt = xst.tile([P, K2], f32, tag="xst",
                                  name=f"xt{mb}_{h}")
                    nc.sync.dma_start(
                        xt[:], xap[mb * P:(mb + 1) * P, h * K2:(h + 1) * K2])
                    x_stage[(mb, h)] = xt

            # -------- weight loads (SP queue, lookahead) --------
            w_stage = {}

            def wload(oc):
                tiles = []
                for h in range(2):
                    wh = wst.tile([P, K2], f32, tag="wst",
                                  name=f"wh{oc}_{h}")
                    nc.sync.dma_start(
                        wh[:], wap[oc * P:(oc + 1) * P, h * K2:(h + 1) * K2])
                    tiles.append(wh)
                w_stage[oc] = tiles

            for _oc in range(min(2, OC)):
                wload(_oc)

            # -------- activation quantization + transpose --------
            xq_t = xq_pool.tile([P, KSUB, M], f16, name="xq_t")
            for mb in range(MB):
                for h in range(2):
                    xt = x_stage.pop((mb, h))
                    xg = xt.rearrange("p (g e) -> p g e", e=G)
                    am = small.tile([P, NGX], f32, tag="am")
                    nc.vector.tensor_reduce(am[:], xg, Ax.X, Alu.max,
                                            apply_absolute_value=True)
                    am2 = small.tile([P, NGX], f32, tag="am2")
                    nc.vector.tensor_scalar(am2[:], am[:], EPS, None, Alu.max)
                    rc = small.tile([P, NGX], f32, tag="rc")
                    nc.vector.reciprocal(rc[:], am2[:])
                    scale = small.tile([P, NGX], f32, tag="scale")
                    nc.vector.tensor_scalar(scale[:], rc[:], QMAX, None,
                                            Alu.mult)
                    inv = small.tile([P, NGX], f32, tag="inv")
                    nc.vector.tensor_scalar(inv[:], am2[:], INV_QMAX, None,
                                            Alu.mult)
                    # xs = x * scale (group-broadcast)
                    nc.vector.tensor_tensor(
                        xg, xg,
                        scale[:, :, None].to_broadcast((P, NGX, G)),
                        Alu.mult)
                    # q = rint(xs) via magic add/sub
                    nc.vector.tensor_scalar(xt[:], xt[:], MAGIC, MAGIC,
                                            Alu.add, Alu.subtract)
                    # x_q = q * (absmax/127) -> fp16
                    xq16 = xq16_pool.tile([P, K2], f16, tag="xq16",
                                          name=f"xq16_{mb}_{h}")
                    nc.vector.tensor_tensor(
                        xq16.rearrange("p (g e) -> p g e", e=G), xg,
                        inv[:, :, None].to_broadcast((P, NGX, G)),
                        Alu.mult)
                    nc.sync.dma_start_transpose(
                        xq_t[:, h * KS2:(h + 1) * KS2,
                             mb * P:(mb + 1) * P], xq16[:])

            # -------- weight ternarize --------
            wt_tiles = {}
            bp_tiles = {}

            def wternarize(oc):
                whs = w_stage.pop(oc)
                gs = small.tile([P, NGW], f32, tag="gs")
                for h in range(2):
                    nc.vector.tensor_reduce(
                        gs[:, h * NGX:(h + 1) * NGX],
                        whs[h].rearrange("p (g e) -> p g e", e=G),
                        Ax.X, Alu.add, apply_absolute_value=True)
                # s = max(mean|row|, eps), two-stage compensated sum
                hq = small.tile([P, NGW], f32, tag="hq")
                nc.vector.tensor_scalar(hq[:], gs[:], MAGIC2, MAGIC2,
                                        Alu.add, Alu.subtract)
                lq = small.tile([P, NGW], f32, tag="lq")
                nc.vector.tensor_tensor(lq[:], gs[:], hq[:], Alu.subtract)
                sh = small.tile([P, 1], f32, tag="sh")
                nc.vector.tensor_reduce(sh[:], hq[:], Ax.X, Alu.add)
                sl = small.tile([P, 1], f32, tag="sl")
                nc.vector.tensor_reduce(sl[:], lq[:], Ax.X, Alu.add)
                ssum = small.tile([P, 1], f32, tag="ssum")
                nc.vector.tensor_tensor(ssum[:], sh[:], sl[:], Alu.add)
                sv = small.tile([P, 1], f32, tag="svv")
                nc.vector.tensor_scalar(sv[:], ssum[:],
                                        float(np.float32(1.0 / K)),
                                        EPS, Alu.mult, Alu.max)
                # eviction scale is 0.5*s (the sign-sum below is 2t)
                bp = sv_pool.tile([P, 1], f32, tag="bp", name=f"bp{oc}")
                nc.vector.tensor_scalar(bp[:], sv[:], 0.5, None, Alu.mult)
                bp_tiles[oc] = bp
                bn = small.tile([P, 1], f32, tag="bn")
                nc.vector.tensor_scalar(bn[:], sv[:], -0.5, None, Alu.mult)
                # 2t = sign(w-0.5s) + sign(w+0.5s); exact comparisons.
                # Boundary |w|==0.5s gives +-1; the f32 grid-of-2 magic
                # round maps it to 0 (= round-half-even of w/s).
                wt = wt_pool.tile([P, KSUB, P], f16, tag="wt",
                                  name=f"wt{oc}")
                wt_tiles[oc] = wt
                for h in range(2):
                    sga = sg_pool.tile([P, K2], f16, tag="sga",
                                       name=f"sga{oc}_{h}")
                    nc.scalar.activation(out=sga[:], in_=whs[h][:],
                                         func=Act.Sign, bias=bn[:],
                                         scale=1.0)
                    sgb = sg_pool.tile([P, K2], f16, tag="sgb",
                                       name=f"sgb{oc}_{h}")
                    nc.scalar.activation(out=sgb[:], in_=whs[h][:],
                                         func=Act.Sign, bias=bp[:],
                                         scale=1.0)
                    nc.vector.tensor_tensor(sga[:], sga[:], sgb[:], Alu.add)
                    nc.vector.tensor_scalar(sga[:], sga[:], FIXC, FIXC,
                                            Alu.add, Alu.subtract)
                    nc.sync.dma_start_transpose(
                        wt[:, h * KS2:(h + 1) * KS2, :], sga[:])

            # -------- matmul + eviction --------
            for oc in range(OC):
                if oc + 2 < OC:
                    wload(oc + 2)
                wternarize(oc)
                wt = wt_tiles.pop(oc)
                bp = bp_tiles.pop(oc)
                for mh in range(MH):
                    ps = ps_pool.tile([P, 512], f32)
                    for ks in range(KSUB):
                        nc.tensor.matmul(
                            ps[:], wt[:, ks, :],
                            xq_t[:, ks, mh * 512:(mh + 1) * 512],
                            start=(ks == 0), stop=(ks == KSUB - 1))
                    ev = ev_pool.tile([P, 512], f32)
                    nc.scalar.activation(out=ev[:], in_=ps[:],
                                         func=Act.Copy, scale=bp[:])
                    nc.gpsimd.dma_start(
                        oap[oc * P:(oc + 1) * P, mh * 512:(mh + 1) * 512],
                        ev[:])

    nc.compile()
    return nc


def _get_nc():
    if "nc" not in _cache:
        _cache["nc"] = _build(M_C, D_IN, D_OUT)
    return _cache["nc"]


def run(x, weight, trace=False):
    """Run on 8 NeuronCores; returns (full output [B,S,D_OUT], results obj)."""
    from concourse.bass_utils import run_bass_kernel_spmd

    x = np.ascontiguousarray(np.asarray(x, dtype=np.float32))
    w = np.ascontiguousarray(np.asarray(weight, dtype=np.float32))
    assert x.shape == (B, S, D_IN) and w.shape == (D_OUT, D_IN)
    xf = x.reshape(M_TOT, D_IN)
    nc = _get_nc()
    in_maps = [
        {"x": np.ascontiguousarray(xf[c * M_C:(c + 1) * M_C]), "w": w}
        for c in range(N_CORES)
    ]
    res = run_bass_kernel_spmd(nc, in_maps, core_ids=list(range(N_CORES)),
                               trace=trace)
    outf = np.concatenate(
        [res.results[c]["out"].T for c in range(N_CORES)], axis=0)
    return np.ascontiguousarray(outf).reshape(B, S, D_OUT), res


def kernel(x, weight):
    out, _ = run(x, weight)
    return out
